# revision 42
# baseline (speedup 1.0000x reference)
"""Causal self-attention on 8 trn2 NeuronCores.

Sharding: core = (batch b, head-group g) with b in 0..3, g in 0..1.
Each core computes, for its batch and its 8 heads (512 of 1024 embed dims):
  QT/KT projections stored transposed [e', s] (e' on partitions)
  V stored [s, e'] with a ones-column appended per head
  S^T[k, q] = K_h Q_h^T      (scores transposed; k on partitions)
  P^T = exp(S^T / 8)         (no max-subtraction; scores are O(1))
  causal zeroing of P^T via gpsimd affine_select on the 128-col
  triangle subtile of diagonal tiles
  att'^T[d, q] = sum_k V'_h[k, d] P^T[k, q]   (row 64 = softmax denom l)
  att_n^T = att'^T[0:64] * (1/l)  (gpsimd partition_broadcast of 1/l)
  out_partial = att_n^T.T @ Wo[rows_g, :]
Host sums the two g-partials per batch.

All matmuls run in bfloat16.  x is cast to bf16 on the host and kept
resident in SBUF, so each x tile is DMA'd exactly once.

Scheduling: the attention inner loop is Scalar-bound (2 exps per kt
step cost more than the 4 matmuls), so projection / output-projection
matmul groups are interleaved INTO the attention kt stream by a
deficit-paced scheduler that models PE and ACT clocks.  This keeps the
Tensor engine saturated (and its DVFS state at full clock) instead of
stalling on exp.  The AV matmul runs one kt behind its score matmul
(software pipelining) so the in-order PE stream never blocks on a
just-issued exp.
"""
import sys

if "/opt/trn_rl_repo" not in sys.path:
    sys.path.insert(0, "/opt/trn_rl_repo")

import numpy as np
import ml_dtypes

import concourse.bacc as bacc
import concourse.mybir as mybir
import concourse.tile as tile
from concourse.bass_utils import run_bass_kernel_spmd

S = 2048          # sequence length
E = 1024          # embed dim
G = 512           # per-core head-group width (8 heads x 64)
HD = 64           # head dim
NH = 8            # heads per core
EC = E // 128     # 8 E-chunks
ST = S // 128     # 16 s-tiles
SB = S // 512     # 4 s-blocks
F32 = mybir.dt.float32
BF16 = mybir.dt.bfloat16
EXP = mybir.ActivationFunctionType.Exp
GE = mybir.AluOpType.is_ge

_CACHE = {}


def _emit(nc, tc):
    xT = nc.declare_dram_parameter("xT", [EC, 128, S], BF16, isOutput=False)
    wq = nc.declare_dram_parameter("wq", [4, 128, EC, 128], BF16,
                                   isOutput=False)
    wk = nc.declare_dram_parameter("wk", [4, 128, EC, 128], BF16,
                                   isOutput=False)
    wv = nc.declare_dram_parameter("wv", [E, G], BF16, isOutput=False)
    wo = nc.declare_dram_parameter("wo", [G, E], BF16, isOutput=False)
    c_ones = nc.declare_dram_parameter("c_ones", [128, NH], BF16,
                                       isOutput=False)
    out = nc.declare_dram_parameter("out", [S, E], F32, isOutput=True)

    # ---- long-lived SBUF state ----
    persist1 = tc.alloc_tile_pool(name="persist1", bufs=1, side="right")
    xb = [persist1.tile([128, S], BF16, name=f"xb{ec}", tag=f"xb{ec}")
          for ec in range(EC)]
    qT_sb, kT_sb = [], []
    for c in range(4):
        qT_sb.append(persist1.tile([128, S], BF16, name=f"qT{c}",
                                   tag=f"qT{c}"))
        kT_sb.append(persist1.tile([128, S], BF16, name=f"kT{c}",
                                   tag=f"kT{c}"))
    vP = [persist1.tile([128, NH, HD + 1], BF16, name=f"vP{st}",
                        tag=f"vP{st}") for st in range(ST)]
    att_n = [persist1.tile([128, S], BF16, name=f"attn{c}", tag=f"attn{c}")
             for c in range(4)]
    ones_sb = persist1.tile([128, NH], BF16, name="ones_sb", tag="ones_sb")
    wqk_sb = {}
    for c in range(4):
        wqk_sb[("q", c)] = persist1.tile([128, EC, 128], BF16,
                                         name=f"wq{c}", tag=f"wq{c}")
        wqk_sb[("k", c)] = persist1.tile([128, EC, 128], BF16,
                                         name=f"wk{c}", tag=f"wk{c}")
    wv_t = [persist1.tile([128, G], BF16, name=f"wv{ec}", tag=f"wv{ec}")
            for ec in range(EC)]
    wo_sb = [persist1.tile([128, E], BF16, name=f"wo{c}", tag=f"wo{c}")
             for c in range(4)]

    # ---- input DMAs, ordered so early compute unblocks first ----
    for wname, wdram in (("q", wq), ("k", wk)):
        nc.sync.dma_start(out=wqk_sb[(wname, 0)], in_=wdram[0])
    for ec in range(EC):     # x s-block 0 (first QK/V groups need it)
        nc.sync.dma_start(out=xb[ec][:, 0:512], in_=xT[ec][:, 0:512])
    for ec in range(EC):
        nc.sync.dma_start(out=wv_t[ec], in_=wv[ec * 128:(ec + 1) * 128, :])
    nc.sync.dma_start(out=ones_sb, in_=c_ones[:, :])
    for sb_i in range(1, SB):
        for ec in range(EC):
            nc.sync.dma_start(out=xb[ec][:, sb_i * 512:(sb_i + 1) * 512],
                              in_=xT[ec][:, sb_i * 512:(sb_i + 1) * 512])
    for c in range(1, 4):
        for wname, wdram in (("q", wq), ("k", wk)):
            nc.sync.dma_start(out=wqk_sb[(wname, c)], in_=wdram[c])
    for c in range(4):
        nc.sync.dma_start(out=wo_sb[c], in_=wo[c * 128:(c + 1) * 128, :])

    # ---- PSUM pools: 3 + 3 + 2 = 8 banks ----
    pst = tc.alloc_tile_pool(name="pst", bufs=3, space="PSUM")
    psatt = tc.alloc_tile_pool(name="psatt", bufs=3, space="PSUM")
    pp = tc.alloc_tile_pool(name="pp", bufs=2, space="PSUM")
    ptp = tc.alloc_tile_pool(name="ptp", bufs=8)
    smalls = tc.alloc_tile_pool(name="smalls", bufs=2)
    ostage = tc.alloc_tile_pool(name="ostage", bufs=2)

    # ---- stall-driven scheduler ----
    # T["pe"] / T["act"] model the two engines' busy-until times over the
    # emitted in-order streams.  Filler (projection / output-projection
    # half-groups, ~880ns of PE work each) is emitted exactly where the
    # in-order PE stream would otherwise stall waiting for an exp.
    T = {"pe": 0.0, "act": 0.0}
    emitted = set()
    filler = []   # list of (uid, closure)

    def _mm(rows):
        T["pe"] += rows * 0.43

    def _act(width):
        T["act"] = max(T["act"], T["pe"]) + width * 0.833 + 260.0

    def _emit_next_filler():
        uid, fn = filler.pop(0)
        fn()
        emitted.add(uid)

    def flush_for(need):
        while need - emitted:
            _emit_next_filler()

    def fill_until(t):
        while T["pe"] < t and filler:
            _emit_next_filler()

    def _proj_halves(uid, lhs_of_ec, rhs_of_ec, finish):
        # one [128,512] psum accumulation group as two filler halves
        state = {}

        def fn_a():
            state["ps"] = pp.tile([128, 512], F32, name="ps_proj",
                                  tag="ps_proj")
            for ec in range(4):
                nc.tensor.matmul(
                    state["ps"], lhsT=lhs_of_ec(ec), rhs=rhs_of_ec(ec),
                    start=(ec == 0), stop=False, skip_group_check=True)
            _mm(4 * 512)

        def fn_b():
            for ec in range(4, EC):
                nc.tensor.matmul(
                    state["ps"], lhsT=lhs_of_ec(ec), rhs=rhs_of_ec(ec),
                    start=False, stop=(ec == EC - 1), skip_group_check=True)
            _mm(4 * 512)
            finish(state["ps"])
        return [(uid + ("a",), fn_a), (uid, fn_b)]

    def qk_units(c, sb_i):
        units = []
        for wname, dest in (("q", qT_sb), ("k", kT_sb)):
            def finish(ps, dest=dest):
                nc.vector.tensor_copy(
                    dest[c][:, sb_i * 512:(sb_i + 1) * 512], ps)
            units.extend(_proj_halves(
                ("qk", c, sb_i, wname),
                lambda ec, wname=wname: wqk_sb[(wname, c)][:, ec, :],
                lambda ec: xb[ec][:, sb_i * 512:(sb_i + 1) * 512],
                finish))
        return units

    def v_units(st):
        def finish(ps):
            nc.vector.tensor_copy(vP[st][:, :, 0:HD],
                                  ps.rearrange("p (h d) -> p h d", h=NH))
            nc.vector.tensor_copy(vP[st][:, :, HD], ones_sb)
        return _proj_halves(
            ("v", st),
            lambda ec: xb[ec][:, st * 128:(st + 1) * 128],
            lambda ec: wv_t[ec],
            finish)

    def o_units(qb, s4):
        st = qb * 4 + s4
        units = []
        for eb in range(2):
            def fn(eb=eb):
                ps = pp.tile([128, 512], F32, name="ps_o", tag="ps_proj")
                for c in range(4):
                    nc.tensor.matmul(
                        ps,
                        lhsT=att_n[c][:, st * 128:(st + 1) * 128],
                        rhs=wo_sb[c][:, eb * 512:(eb + 1) * 512],
                        start=(c == 0), stop=(c == 3),
                        skip_group_check=True)
                _mm(4 * 512)
                o_sb = ostage.tile([128, 512], F32, name="o_sb", tag="o_sb")
                nc.vector.tensor_copy(o_sb, ps)
                nc.sync.dma_start(
                    out=out[st * 128:(st + 1) * 128,
                            eb * 512:(eb + 1) * 512],
                    in_=o_sb)
            units.append((("o", qb, s4, eb), fn))
        return units

    for args in [("qk", 0, 0), ("v", 0), ("v", 1), ("v", 2), ("v", 3),
                 ("qk", 1, 0), ("qk", 0, 1),
                 ("v", 4), ("v", 5), ("v", 6), ("v", 7),
                 ("qk", 2, 0), ("qk", 1, 1), ("qk", 0, 2),
                 ("v", 8), ("v", 9), ("v", 10), ("v", 11),
                 ("qk", 3, 0), ("qk", 2, 1), ("qk", 1, 2), ("qk", 0, 3),
                 ("v", 12), ("v", 13), ("v", 14), ("v", 15),
                 ("qk", 3, 1), ("qk", 2, 2), ("qk", 1, 3),
                 ("qk", 3, 2), ("qk", 2, 3), ("qk", 3, 3)]:
        if args[0] == "qk":
            filler.extend(qk_units(args[1], args[2]))
        else:
            filler.extend(v_units(args[1]))

    def attention_block(c, qb):
        flush_for({("qk", c, s, w) for s in range(qb + 1)
                   for w in ("q", "k")})
        last_kt = 4 * qb + 3
        att_ps = [psatt.tile([HD + 1, 512], F32, name="att_ps",
                             tag="att_ps") for _ in range(2)]

        def av(kt, pts, cs, ready):
            flush_for({("v", kt)})
            # the AV matmuls can't start before their exp (+ causal mask)
            # lands; spend filler to keep the in-order PE stream busy
            fill_until(ready)
            for u in range(2):
                nc.tensor.matmul(
                    att_ps[u][:, cs:512],
                    lhsT=vP[kt][:, 2 * c + u, :],
                    rhs=pts[u][:, cs:512],
                    start=(kt == 0), stop=(kt == last_kt),
                    skip_group_check=True)
                _mm(512 - cs)

        pending = None
        for kt in range(last_kt + 1):
            diag = kt >= 4 * qb
            cs = 128 * kt - 512 * qb if diag else 0
            pts = []
            for u in range(2):
                po = u * HD
                s_ps = pst.tile([128, 512], F32, name="s_ps", tag="s_ps")
                nc.tensor.matmul(
                    s_ps[:, cs:512],
                    lhsT=kT_sb[c][po:po + HD, kt * 128:(kt + 1) * 128],
                    rhs=qT_sb[c][po:po + HD, qb * 512 + cs:(qb + 1) * 512],
                    start=True, stop=True, skip_group_check=True,
                    tile_position=(po, 0))
                _mm(512 - cs)
                pt = ptp.tile([128, 512], BF16, name="pt", tag="pt")
                nc.scalar.activation(
                    pt[:, cs:512], s_ps[:, cs:512], EXP, scale=0.125)
                _act(512 - cs)
                if diag:
                    # zero invalid (k > q) inside the 128-col triangle
                    # subtile; columns beyond it are fully valid.
                    # valid iff (512*qb + cs + y) - (128*kt + x) >= 0,
                    # and 512*qb + cs - 128*kt == 0 on the diagonal
                    nc.gpsimd.affine_select(
                        out=pt[:, cs:cs + 128], in_=pt[:, cs:cs + 128],
                        compare_op=GE, fill=0.0,
                        base=0, channel_multiplier=-1,
                        pattern=[[1, 128]])
                pts.append(pt)
            ready = T["act"] + (500.0 if diag else 100.0)
            if pending is not None:
                av(*pending)
            pending = (kt, pts, cs, ready)
        av(*pending)
        for u in range(2):
            po = u * HD
            l_sb = smalls.tile([1, 512], F32, name="l_sb", tag="l_sb")
            nc.vector.tensor_copy(l_sb, att_ps[u][HD:HD + 1, :])
            r_sb = smalls.tile([1, 512], F32, name="r_sb", tag="r_sb")
            nc.vector.reciprocal_approx_fast(out=r_sb, in_=l_sb)
            rb_sb = smalls.tile([HD, 512], F32, name="rb_sb", tag="rb_sb")
            nc.gpsimd.partition_broadcast(rb_sb, r_sb)
            nc.vector.tensor_mul(
                att_n[c][po:po + HD, qb * 512:(qb + 1) * 512],
                att_ps[u][0:HD, :], rb_sb)
        fill_until(T["act"])

    # ---- wavefront over anti-diagonals with paced filler ----
    done_qb = [0, 0, 0, 0]
    for d in range(7):
        for cc in range(3, -1, -1):
            qb = d - cc
            if not (0 <= qb <= 3):
                continue
            attention_block(cc, qb)
            done_qb[qb] += 1
            if done_qb[qb] == 4:
                for s4 in range(4):
                    filler.extend(o_units(qb, s4))
    while filler:
        _emit_next_filler()

    ostage.release()
    smalls.release()
    ptp.release()
    pp.release()
    psatt.release()
    pst.release()
    persist1.release()


def _build():
    if "nc" in _CACHE:
        return _CACHE["nc"]
    nc = bacc.Bacc()
    with tile.TileContext(nc) as tc:
        _emit(nc, tc)
    nc.compile()
    _CACHE["nc"] = nc
    return nc


def _bf16(a):
    return np.ascontiguousarray(a.astype(ml_dtypes.bfloat16))


def _pack_w(Wg):
    # [E, G] -> [pair c, partition p, ec, col m]:
    # out[c, p, ec, m] = Wg[ec*128 + p, c*128 + m]
    return np.ascontiguousarray(
        Wg.reshape(EC, 128, 4, 128).transpose(2, 1, 0, 3))


def _make_in_maps(inputs):
    x = np.asarray(inputs["x"], dtype=np.float32)
    Wq = np.asarray(inputs["Wq"], dtype=np.float32)
    Wk = np.asarray(inputs["Wk"], dtype=np.float32)
    Wv = np.asarray(inputs["Wv"], dtype=np.float32)
    Wo = np.asarray(inputs["Wo"], dtype=np.float32)
    in_maps = []
    for core in range(8):
        b, g = core // 2, core % 2
        cols = slice(g * G, (g + 1) * G)
        in_maps.append({
            "xT": _bf16(x[b].T.reshape(EC, 128, S)),
            "wq": _bf16(_pack_w(Wq[:, cols])),
            "wk": _bf16(_pack_w(Wk[:, cols])),
            "wv": _bf16(Wv[:, cols]),
            "wo": _bf16(Wo[cols, :]),
            "c_ones": np.ones((128, NH), dtype=ml_dtypes.bfloat16),
        })
    return in_maps


def kernel(x, Wq, Wk, Wv, Wo):
    nc = _build()
    in_maps = _make_in_maps(dict(x=x, Wq=Wq, Wk=Wk, Wv=Wv, Wo=Wo))
    res = run_bass_kernel_spmd(nc, in_maps, core_ids=list(range(8)))
    out = np.zeros((4, S, E), dtype=np.float32)
    for core in range(8):
        out[core // 2] += res.results[core]["out"]
    return out


if __name__ == "__main__":
    rng = np.random.default_rng(0)
    x = rng.standard_normal((4, S, E), dtype=np.float32)
    sc = 1.0 / np.sqrt(E)
    Wq = rng.standard_normal((E, E), dtype=np.float32) * sc
    Wk = rng.standard_normal((E, E), dtype=np.float32) * sc
    Wv = rng.standard_normal((E, E), dtype=np.float32) * sc
    Wo = rng.standard_normal((E, E), dtype=np.float32) * sc
    o = kernel(x, Wq, Wk, Wv, Wo)
    print("out", o.shape, o.dtype, np.abs(o).mean())


# revision 43
# speedup vs baseline: 1.0208x; 1.0208x over previous
"""Causal self-attention on 8 trn2 NeuronCores.

Sharding: core = (batch b, head-group g) with b in 0..3, g in 0..1.
Each core computes, for its batch and its 8 heads (512 of 1024 embed dims):
  QT/KT projections stored transposed [e', s] (e' on partitions)
  V stored [s, e'] with a ones-column appended per head
  S^T[k, q] = K_h Q_h^T      (scores transposed; k on partitions)
  P^T = exp(S^T / 8)         (no max-subtraction; scores are O(1))
  causal zeroing of P^T via gpsimd affine_select on the 128-col
  triangle subtile of diagonal tiles
  att'^T[d, q] = sum_k V'_h[k, d] P^T[k, q]   (row 64 = softmax denom l)
  att_n^T = att'^T[0:64] * (1/l)  (gpsimd partition_broadcast of 1/l)
  out_partial = att_n^T.T @ Wo[rows_g, :]
Host sums the two g-partials per batch.

All matmuls run in bfloat16.  x is cast to bf16 on the host and kept
resident in SBUF, so each x tile is DMA'd exactly once.

Scheduling: the attention inner loop is Scalar-bound (2 exps per kt
step cost more than the 4 matmuls), so projection / output-projection
matmul groups are interleaved INTO the attention kt stream by a
deficit-paced scheduler that models PE and ACT clocks.  This keeps the
Tensor engine saturated (and its DVFS state at full clock) instead of
stalling on exp.  The AV matmul runs one kt behind its score matmul
(software pipelining) so the in-order PE stream never blocks on a
just-issued exp.
"""
import sys

if "/opt/trn_rl_repo" not in sys.path:
    sys.path.insert(0, "/opt/trn_rl_repo")

import numpy as np
import ml_dtypes

import concourse.bacc as bacc
import concourse.mybir as mybir
import concourse.tile as tile
from concourse.bass_utils import run_bass_kernel_spmd

S = 2048          # sequence length
E = 1024          # embed dim
G = 512           # per-core head-group width (8 heads x 64)
HD = 64           # head dim
NH = 8            # heads per core
EC = E // 128     # 8 E-chunks
ST = S // 128     # 16 s-tiles
SB = S // 512     # 4 s-blocks
F32 = mybir.dt.float32
BF16 = mybir.dt.bfloat16
EXP = mybir.ActivationFunctionType.Exp
GE = mybir.AluOpType.is_ge

_CACHE = {}


def _emit(nc, tc):
    xT = nc.declare_dram_parameter("xT", [EC, 128, S], BF16, isOutput=False)
    wq = nc.declare_dram_parameter("wq", [4, 128, EC, 128], BF16,
                                   isOutput=False)
    wk = nc.declare_dram_parameter("wk", [4, 128, EC, 128], BF16,
                                   isOutput=False)
    wv = nc.declare_dram_parameter("wv", [E, G], BF16, isOutput=False)
    wo = nc.declare_dram_parameter("wo", [G, E], BF16, isOutput=False)
    c_ones = nc.declare_dram_parameter("c_ones", [128, NH], BF16,
                                       isOutput=False)
    out = nc.declare_dram_parameter("out", [S, E], F32, isOutput=True)

    # ---- long-lived SBUF state ----
    persist1 = tc.alloc_tile_pool(name="persist1", bufs=1, side="right")
    xb = [persist1.tile([128, S], BF16, name=f"xb{ec}", tag=f"xb{ec}")
          for ec in range(EC)]
    qT_sb, kT_sb = [], []
    for c in range(4):
        qT_sb.append(persist1.tile([128, S], BF16, name=f"qT{c}",
                                   tag=f"qT{c}"))
        kT_sb.append(persist1.tile([128, S], BF16, name=f"kT{c}",
                                   tag=f"kT{c}"))
    vP = [persist1.tile([128, NH, HD + 1], BF16, name=f"vP{st}",
                        tag=f"vP{st}") for st in range(ST)]
    att_n = [persist1.tile([128, S], BF16, name=f"attn{c}", tag=f"attn{c}")
             for c in range(4)]
    ones_sb = persist1.tile([128, NH], BF16, name="ones_sb", tag="ones_sb")
    wqk_sb = {}
    for c in range(4):
        wqk_sb[("q", c)] = persist1.tile([128, EC, 128], BF16,
                                         name=f"wq{c}", tag=f"wq{c}")
        wqk_sb[("k", c)] = persist1.tile([128, EC, 128], BF16,
                                         name=f"wk{c}", tag=f"wk{c}")
    wv_t = [persist1.tile([128, G], BF16, name=f"wv{ec}", tag=f"wv{ec}")
            for ec in range(EC)]
    wo_sb = [persist1.tile([128, E], BF16, name=f"wo{c}", tag=f"wo{c}")
             for c in range(4)]

    # ---- input DMAs, ordered so early compute unblocks first ----
    for wname, wdram in (("q", wq), ("k", wk)):
        nc.sync.dma_start(out=wqk_sb[(wname, 0)], in_=wdram[0])
    for ec in range(EC):     # x s-block 0 (first QK/V groups need it)
        nc.sync.dma_start(out=xb[ec][:, 0:512], in_=xT[ec][:, 0:512])
    for ec in range(EC):
        nc.sync.dma_start(out=wv_t[ec], in_=wv[ec * 128:(ec + 1) * 128, :])
    nc.sync.dma_start(out=ones_sb, in_=c_ones[:, :])
    for sb_i in range(1, SB):
        for ec in range(EC):
            nc.sync.dma_start(out=xb[ec][:, sb_i * 512:(sb_i + 1) * 512],
                              in_=xT[ec][:, sb_i * 512:(sb_i + 1) * 512])
    for c in range(1, 4):
        for wname, wdram in (("q", wq), ("k", wk)):
            nc.sync.dma_start(out=wqk_sb[(wname, c)], in_=wdram[c])
    for c in range(4):
        nc.sync.dma_start(out=wo_sb[c], in_=wo[c * 128:(c + 1) * 128, :])

    # ---- PSUM pools: 3 + 3 + 2 = 8 banks ----
    pst = tc.alloc_tile_pool(name="pst", bufs=3, space="PSUM")
    psatt = tc.alloc_tile_pool(name="psatt", bufs=3, space="PSUM")
    pp = tc.alloc_tile_pool(name="pp", bufs=2, space="PSUM")
    ptp = tc.alloc_tile_pool(name="ptp", bufs=8)
    smalls = tc.alloc_tile_pool(name="smalls", bufs=2)
    ostage = tc.alloc_tile_pool(name="ostage", bufs=2)

    # ---- stall-driven scheduler ----
    # T["pe"] / T["act"] model the two engines' busy-until times over the
    # emitted in-order streams.  Filler (projection / output-projection
    # half-groups, ~880ns of PE work each) is emitted exactly where the
    # in-order PE stream would otherwise stall waiting for an exp.
    T = {"pe": 0.0, "act": 0.0}
    emitted = set()
    filler = []   # list of (uid, closure)

    def _mm(rows):
        T["pe"] += rows * 0.43

    def _act(width):
        T["act"] = max(T["act"], T["pe"]) + width * 0.833 + 190.0

    def _emit_next_filler():
        uid, fn = filler.pop(0)
        fn()
        emitted.add(uid)

    def flush_for(need):
        while need - emitted:
            _emit_next_filler()

    def fill_until(t):
        while T["pe"] < t and filler:
            _emit_next_filler()

    def _proj_halves(uid, lhs_of_ec, rhs_of_ec, finish):
        # one [128,512] psum accumulation group as two filler halves
        state = {}

        def fn_a():
            state["ps"] = pp.tile([128, 512], F32, name="ps_proj",
                                  tag="ps_proj")
            for ec in range(4):
                nc.tensor.matmul(
                    state["ps"], lhsT=lhs_of_ec(ec), rhs=rhs_of_ec(ec),
                    start=(ec == 0), stop=False, skip_group_check=True)
            _mm(4 * 512)

        def fn_b():
            for ec in range(4, EC):
                nc.tensor.matmul(
                    state["ps"], lhsT=lhs_of_ec(ec), rhs=rhs_of_ec(ec),
                    start=False, stop=(ec == EC - 1), skip_group_check=True)
            _mm(4 * 512)
            finish(state["ps"])
        return [(uid + ("a",), fn_a), (uid, fn_b)]

    def qk_units(c, sb_i):
        units = []
        for wname, dest in (("q", qT_sb), ("k", kT_sb)):
            def finish(ps, dest=dest):
                nc.vector.tensor_copy(
                    dest[c][:, sb_i * 512:(sb_i + 1) * 512], ps)
            units.extend(_proj_halves(
                ("qk", c, sb_i, wname),
                lambda ec, wname=wname: wqk_sb[(wname, c)][:, ec, :],
                lambda ec: xb[ec][:, sb_i * 512:(sb_i + 1) * 512],
                finish))
        return units

    def v_units(st):
        def finish(ps):
            nc.vector.tensor_copy(vP[st][:, :, 0:HD],
                                  ps.rearrange("p (h d) -> p h d", h=NH))
            nc.vector.tensor_copy(vP[st][:, :, HD], ones_sb)
        return _proj_halves(
            ("v", st),
            lambda ec: xb[ec][:, st * 128:(st + 1) * 128],
            lambda ec: wv_t[ec],
            finish)

    def o_units(qb, s4):
        st = qb * 4 + s4
        units = []
        for eb in range(2):
            def fn(eb=eb):
                ps = pp.tile([128, 512], F32, name="ps_o", tag="ps_proj")
                for c in range(4):
                    nc.tensor.matmul(
                        ps,
                        lhsT=att_n[c][:, st * 128:(st + 1) * 128],
                        rhs=wo_sb[c][:, eb * 512:(eb + 1) * 512],
                        start=(c == 0), stop=(c == 3),
                        skip_group_check=True)
                _mm(4 * 512)
                o_sb = ostage.tile([128, 512], F32, name="o_sb", tag="o_sb")
                nc.vector.tensor_copy(o_sb, ps)
                nc.sync.dma_start(
                    out=out[st * 128:(st + 1) * 128,
                            eb * 512:(eb + 1) * 512],
                    in_=o_sb)
            units.append((("o", qb, s4, eb), fn))
        return units

    for args in [("qk", 0, 0), ("v", 0), ("v", 1), ("v", 2), ("v", 3),
                 ("qk", 1, 0), ("qk", 0, 1),
                 ("v", 4), ("v", 5), ("v", 6), ("v", 7),
                 ("qk", 2, 0), ("qk", 1, 1), ("qk", 0, 2),
                 ("v", 8), ("v", 9), ("v", 10), ("v", 11),
                 ("qk", 3, 0), ("qk", 2, 1), ("qk", 1, 2), ("qk", 0, 3),
                 ("v", 12), ("v", 13), ("v", 14), ("v", 15),
                 ("qk", 3, 1), ("qk", 2, 2), ("qk", 1, 3),
                 ("qk", 3, 2), ("qk", 2, 3), ("qk", 3, 3)]:
        if args[0] == "qk":
            filler.extend(qk_units(args[1], args[2]))
        else:
            filler.extend(v_units(args[1]))

    def attention_block(c, qb):
        flush_for({("qk", c, s, w) for s in range(qb + 1)
                   for w in ("q", "k")})
        last_kt = 4 * qb + 3
        att_ps = [psatt.tile([HD + 1, 512], F32, name="att_ps",
                             tag="att_ps") for _ in range(2)]

        def av(kt, pts, cs, ready):
            flush_for({("v", kt)})
            # the AV matmuls can't start before their exp (+ causal mask)
            # lands; spend filler to keep the in-order PE stream busy
            fill_until(ready)
            for u in range(2):
                nc.tensor.matmul(
                    att_ps[u][:, cs:512],
                    lhsT=vP[kt][:, 2 * c + u, :],
                    rhs=pts[u][:, cs:512],
                    start=(kt == 0), stop=(kt == last_kt),
                    skip_group_check=True)
                _mm(512 - cs)

        pending = None
        for kt in range(last_kt + 1):
            diag = kt >= 4 * qb
            cs = 128 * kt - 512 * qb if diag else 0
            pts = []
            for u in range(2):
                po = u * HD
                s_ps = pst.tile([128, 512], F32, name="s_ps", tag="s_ps")
                nc.tensor.matmul(
                    s_ps[:, cs:512],
                    lhsT=kT_sb[c][po:po + HD, kt * 128:(kt + 1) * 128],
                    rhs=qT_sb[c][po:po + HD, qb * 512 + cs:(qb + 1) * 512],
                    start=True, stop=True, skip_group_check=True,
                    tile_position=(po, 0))
                _mm(512 - cs)
                pt = ptp.tile([128, 512], BF16, name="pt", tag="pt")
                nc.scalar.activation(
                    pt[:, cs:512], s_ps[:, cs:512], EXP, scale=0.125)
                _act(512 - cs)
                if diag:
                    # zero invalid (k > q) inside the 128-col triangle
                    # subtile; columns beyond it are fully valid.
                    # valid iff (512*qb + cs + y) - (128*kt + x) >= 0,
                    # and 512*qb + cs - 128*kt == 0 on the diagonal
                    nc.gpsimd.affine_select(
                        out=pt[:, cs:cs + 128], in_=pt[:, cs:cs + 128],
                        compare_op=GE, fill=0.0,
                        base=0, channel_multiplier=-1,
                        pattern=[[1, 128]])
                pts.append(pt)
            ready = T["act"] + (500.0 if diag else 100.0)
            if pending is not None:
                av(*pending)
            pending = (kt, pts, cs, ready)
        av(*pending)
        for u in range(2):
            po = u * HD
            l_sb = smalls.tile([1, 512], F32, name="l_sb", tag="l_sb")
            nc.vector.tensor_copy(l_sb, att_ps[u][HD:HD + 1, :])
            r_sb = smalls.tile([1, 512], F32, name="r_sb", tag="r_sb")
            nc.vector.reciprocal_approx_fast(out=r_sb, in_=l_sb)
            rb_sb = smalls.tile([HD, 512], F32, name="rb_sb", tag="rb_sb")
            nc.gpsimd.partition_broadcast(rb_sb, r_sb)
            nc.vector.tensor_mul(
                att_n[c][po:po + HD, qb * 512:(qb + 1) * 512],
                att_ps[u][0:HD, :], rb_sb)
        fill_until(T["act"])

    # ---- wavefront over anti-diagonals with paced filler ----
    done_qb = [0, 0, 0, 0]
    for d in range(7):
        for cc in range(3, -1, -1):
            qb = d - cc
            if not (0 <= qb <= 3):
                continue
            attention_block(cc, qb)
            done_qb[qb] += 1
            if done_qb[qb] == 4:
                for s4 in range(4):
                    filler.extend(o_units(qb, s4))
    while filler:
        _emit_next_filler()

    ostage.release()
    smalls.release()
    ptp.release()
    pp.release()
    psatt.release()
    pst.release()
    persist1.release()


def _build():
    if "nc" in _CACHE:
        return _CACHE["nc"]
    nc = bacc.Bacc()
    with tile.TileContext(nc) as tc:
        _emit(nc, tc)
    nc.compile()
    _CACHE["nc"] = nc
    return nc


def _bf16(a):
    return np.ascontiguousarray(a.astype(ml_dtypes.bfloat16))


def _pack_w(Wg):
    # [E, G] -> [pair c, partition p, ec, col m]:
    # out[c, p, ec, m] = Wg[ec*128 + p, c*128 + m]
    return np.ascontiguousarray(
        Wg.reshape(EC, 128, 4, 128).transpose(2, 1, 0, 3))


def _make_in_maps(inputs):
    x = np.asarray(inputs["x"], dtype=np.float32)
    Wq = np.asarray(inputs["Wq"], dtype=np.float32)
    Wk = np.asarray(inputs["Wk"], dtype=np.float32)
    Wv = np.asarray(inputs["Wv"], dtype=np.float32)
    Wo = np.asarray(inputs["Wo"], dtype=np.float32)
    in_maps = []
    for core in range(8):
        b, g = core // 2, core % 2
        cols = slice(g * G, (g + 1) * G)
        in_maps.append({
            "xT": _bf16(x[b].T.reshape(EC, 128, S)),
            "wq": _bf16(_pack_w(Wq[:, cols])),
            "wk": _bf16(_pack_w(Wk[:, cols])),
            "wv": _bf16(Wv[:, cols]),
            "wo": _bf16(Wo[cols, :]),
            "c_ones": np.ones((128, NH), dtype=ml_dtypes.bfloat16),
        })
    return in_maps


def kernel(x, Wq, Wk, Wv, Wo):
    nc = _build()
    in_maps = _make_in_maps(dict(x=x, Wq=Wq, Wk=Wk, Wv=Wv, Wo=Wo))
    res = run_bass_kernel_spmd(nc, in_maps, core_ids=list(range(8)))
    out = np.zeros((4, S, E), dtype=np.float32)
    for core in range(8):
        out[core // 2] += res.results[core]["out"]
    return out


if __name__ == "__main__":
    rng = np.random.default_rng(0)
    x = rng.standard_normal((4, S, E), dtype=np.float32)
    sc = 1.0 / np.sqrt(E)
    Wq = rng.standard_normal((E, E), dtype=np.float32) * sc
    Wk = rng.standard_normal((E, E), dtype=np.float32) * sc
    Wv = rng.standard_normal((E, E), dtype=np.float32) * sc
    Wo = rng.standard_normal((E, E), dtype=np.float32) * sc
    o = kernel(x, Wq, Wk, Wv, Wo)
    print("out", o.shape, o.dtype, np.abs(o).mean())
